# revision 3
# baseline (speedup 1.0000x reference)
"""Trainium2 Bass kernel for nn_CustomConv1D (nealmon-softmax windowed conv).

Computation (reference):
    w = softmax(param5 * i + param6 * i^2),  i = 1..64          # (64,)
    out[b, t, c] = sum_{k<64, ci<10} x[b, 64*t + k, ci] * w[k]  # (256, 512, 10)

x[b] flattened row-major is contiguous f32 and window t occupies 640
consecutive elements, so the job is: for every contiguous 640-element chunk,
compute a weighted sum (weights = repeat(w, 10), channel innermost), then
broadcast that scalar to 10 output channels.

Strategy (pure data-parallel over batch, 8 cores x 32 batches):
  - Contiguous-per-partition split: partition p owns the contiguous
    x_d[p*81920:(p+1)*81920] (128 whole windows), so every per-slab DMA line
    is large AND the per-partition output region is one contiguous 5 KB run
    -> both output stores use >=512 B lines (full DMA rate, no sub-512 B HBM
    read-modify-write).
  - Loads ride the sync HWDGE ring alone (measured at the ~360 GB/s
    HBM-per-core wall); stores + the small weight load ride the scalar ring.
  - Per window, ONE fused DVE instruction (affine_mul_reduce, a registered
    custom DVE op): accum_out = sum(x_window * wfull) does channel-sum, lag
    weighting and lag-sum in a single pass -> fewest DVE cycles and the
    shortest post-last-load drain chain.  ACT broadcasts each window scalar
    to 10 channels into a persistent out block.
  - Slab schedule [5120]*14 + [2560, 3840, 2560, 1280] keeps the DVE at most
    ~4.5 us behind the load stream at the end (HW-tuned shrinking tail).

Measured (For_i slope, axon RPC floor cancelled): pure loads run at the HBM
wall (~120.6 us/iter incl. fixed costs); the full pipeline adds only the
structural drain.  TimelineSim single-shot: 126.6 us vs 129.3 for the
3-op-chain baseline.
"""

import numpy as np

import concourse.bass as bass
import concourse.bacc as bacc
import concourse.mybir as mybir
import concourse.tile as tile
from concourse.bass_utils import run_bass_kernel_spmd

# Problem shape (hardcoded per contract: kernel.py must be self-contained).
B, T, C = 256, 32768, 10
KW = 64
N_CORES = 8
B_PER_CORE = B // N_CORES                      # 32
NWIN = T // KW                                 # 512
ELEMS_PER_CORE = B_PER_CORE * T * C            # 10,485,760
F_TOTAL = ELEMS_PER_CORE // 128                # 81,920 elems per partition
OUT_PER_PART = F_TOTAL // KW                   # 1,280 out elems per partition
OUT_ELEMS_PER_CORE = 128 * OUT_PER_PART        # 163,840

# Slab free sizes (multiples of 640; sum = 81920).  The fused per-window
# DVE op makes DVE instruction count independent of slab partitioning, so a
# long geometrically-decaying tail walks the DVE backlog down to the ~0.8 us
# single-window floor by the last load (decay ratio ~0.9 = DVE/load rate).
SIZES = ([5120] * 6 + [4480] * 2 + [3840] * 2 + [3200] * 2 + [2560] * 3
         + [1920] * 4 + [1280] * 5 + [640] * 10)
# Both store regions are >=512 B per partition line (full DMA rate).  store1
# [0,1120) is complete after slab 13 but issued from the ACT queue after slab
# 16's broadcast so its data does not displace load traffic; store2
# [1120,1280) needs the last slab and goes on the by-then-idle sync ring.
STORE1_END = 1120

XBUFS, SBUFS = 6, 3

_FP32 = mybir.dt.float32

_cache = {}


def _build_bass(reps: int = 1):
    """Build the single-core Bass program (same NEFF runs SPMD on all cores).

    reps > 1 wraps the pipeline in a tc.For_i loop repeating it on the same
    data — used only for slope-based HW timing in test.py.
    """
    nc = bacc.Bacc("TRN2", target_bir_lowering=False, debug=False,
                   num_devices=N_CORES)

    x_d = nc.dram_tensor("x", (ELEMS_PER_CORE,), _FP32, kind="ExternalInput").ap()
    w_d = nc.dram_tensor("w", (128, KW * C), _FP32, kind="ExternalInput").ap()
    out_d = nc.dram_tensor("out", (OUT_ELEMS_PER_CORE,), _FP32,
                           kind="ExternalOutput").ap()

    x128 = x_d.rearrange("(p f) -> p f", p=128)
    o128 = out_d.rearrange("(p f) -> p f", p=128)

    with tile.TileContext(nc) as tc:
        with (
            tc.tile_pool(name="const", bufs=1) as cpool,
            tc.tile_pool(name="x", bufs=XBUFS) as xpool,
            tc.tile_pool(name="sc", bufs=2) as scpool,
            tc.tile_pool(name="s", bufs=SBUFS) as spool,
            tc.tile_pool(name="ob", bufs=2) as obpool,
        ):
            wt = cpool.tile([128, KW * C], _FP32)
            nc.scalar.dma_start(wt[:], w_d)

            def body():
                ob = obpool.tile([128, OUT_PER_PART], _FP32, tag="ob")
                off = 0
                for idx, f in enumerate(SIZES):
                    wpp = f // (KW * C)  # windows in this slab per partition
                    oo = off // KW       # out offset within ob

                    xt = xpool.tile([128, max(SIZES)], _FP32, tag="x")
                    nc.sync.dma_start(xt[:, :f], x128[:, off:off + f])

                    # fused channel-sum + lag-weight + lag-sum, one custom
                    # DVE op per window: accum_out = sum(x_window * wfull)
                    st = spool.tile([128, max(SIZES) // (KW * C)],
                                    _FP32, tag="s")
                    sc = scpool.tile([128, KW * C], _FP32, tag="sc")
                    for t in range(wpp):
                        nc.vector.affine_mul_reduce(
                            sc[:],
                            st[:, t:t + 1],
                            xt[:, t * KW * C:(t + 1) * KW * C],
                            wt[:],
                            1.0, 0.0)

                    # broadcast to 10 channels into the persistent out block
                    nc.scalar.copy(
                        ob[:, oo:oo + wpp * C].rearrange(
                            "p (t c) -> p t c", c=C),
                        st[:, :wpp].unsqueeze(2).broadcast_to([128, wpp, C]))

                    if idx == len(SIZES) - 2:
                        nc.scalar.dma_start(
                            o128[:, 0:STORE1_END], ob[:, 0:STORE1_END])
                    elif idx == len(SIZES) - 1:
                        nc.sync.dma_start(
                            o128[:, STORE1_END:], ob[:, STORE1_END:])
                    off += f

            if reps > 1:
                with tc.For_i(0, reps, 1):
                    body()
            else:
                body()

    nc.compile()
    return nc


def _softmax_w(param5, param6):
    i = np.arange(1, KW + 1, dtype=np.float32)
    ll = np.float32(param5) * i + np.float32(param6) * i * i
    ll = ll - ll.max()
    e = np.exp(ll)
    return (e / e.sum()).astype(np.float32)


def _weights(param5, param6):
    wfull = np.repeat(_softmax_w(param5, param6), C)      # (640,) = w[j//10]
    return np.tile(wfull[None, :], (128, 1)).copy()       # (128, 640)


def kernel(x: np.ndarray, param5: np.ndarray, param6: np.ndarray):
    x = np.ascontiguousarray(x, dtype=np.float32)
    assert x.shape == (B, T, C)

    if "nc" not in _cache:
        _cache["nc"] = _build_bass()
    nc = _cache["nc"]

    w_tiled = _weights(param5, param6)
    shards = x.reshape(N_CORES, ELEMS_PER_CORE)
    in_maps = [{"x": shards[c], "w": w_tiled} for c in range(N_CORES)]

    res = run_bass_kernel_spmd(nc, in_maps, core_ids=list(range(N_CORES)))
    _cache["last_results"] = res

    out = np.empty((B, NWIN, C), dtype=np.float32)
    for c in range(N_CORES):
        out[c * B_PER_CORE:(c + 1) * B_PER_CORE] = (
            res.results[c]["out"].reshape(B_PER_CORE, NWIN, C))
    return out


# revision 6
# speedup vs baseline: 1.0005x; 1.0005x over previous
"""Trainium2 Bass kernel for nn_CustomConv1D (nealmon-softmax windowed conv).

Computation (reference):
    w = softmax(param5 * i + param6 * i^2),  i = 1..64          # (64,)
    out[b, t, c] = sum_{k<64, ci<10} x[b, 64*t + k, ci] * w[k]  # (256, 512, 10)

x[b] flattened row-major is contiguous f32 and window t occupies 640
consecutive elements, so the job is: for every contiguous 640-element chunk,
compute a weighted sum (weights = repeat(w, 10), channel innermost), then
broadcast that scalar to 10 output channels.

Strategy (pure data-parallel over batch, 8 cores x 32 batches):
  - Contiguous-per-partition split: partition p owns the contiguous
    x_d[p*81920:(p+1)*81920] (128 whole windows), so every per-slab DMA line
    is large AND the per-partition output region is one contiguous 5 KB run
    -> both output stores use >=512 B lines (full DMA rate, no sub-512 B HBM
    read-modify-write).
  - Loads ride the sync HWDGE ring alone (measured at the ~360 GB/s
    HBM-per-core wall); stores + the small weight load ride the scalar ring.
  - Per window, ONE fused DVE instruction (affine_mul_reduce, a registered
    custom DVE op): accum_out = sum(x_window * wfull) does channel-sum, lag
    weighting and lag-sum in a single pass -> fewest DVE cycles and the
    shortest post-last-load drain chain.  ACT broadcasts each window scalar
    to 10 channels into a persistent out block.
  - Slab schedule [5120]*14 + [2560, 3840, 2560, 1280] keeps the DVE at most
    ~4.5 us behind the load stream at the end (HW-tuned shrinking tail).

Measured (For_i slope, axon RPC floor cancelled): pure loads run at the HBM
wall (~120.6 us/iter incl. fixed costs); the full pipeline adds only the
structural drain.  TimelineSim single-shot: 126.6 us vs 129.3 for the
3-op-chain baseline.
"""

import numpy as np

import concourse.bass as bass
import concourse.bacc as bacc
import concourse.mybir as mybir
import concourse.tile as tile
from concourse.bass_utils import run_bass_kernel_spmd

# Problem shape (hardcoded per contract: kernel.py must be self-contained).
B, T, C = 256, 32768, 10
KW = 64
N_CORES = 8
B_PER_CORE = B // N_CORES                      # 32
NWIN = T // KW                                 # 512
ELEMS_PER_CORE = B_PER_CORE * T * C            # 10,485,760
F_TOTAL = ELEMS_PER_CORE // 128                # 81,920 elems per partition
OUT_PER_PART = F_TOTAL // KW                   # 1,280 out elems per partition
OUT_ELEMS_PER_CORE = 128 * OUT_PER_PART        # 163,840

# Slab free sizes (multiples of 640; sum = 81920).  The fused per-window
# DVE op makes DVE instruction count independent of slab partitioning, so a
# long geometrically-decaying tail walks the DVE backlog down to the ~0.8 us
# single-window floor by the last load (decay ratio ~0.9 = DVE/load rate).
SIZES = ([5120] * 6 + [4480] * 2 + [3840] * 2 + [3200] * 2 + [2560] * 3
         + [1920] * 4 + [1280] * 5 + [640] * 10)
# Both store regions are >=512 B per partition line (full DMA rate).  store1
# [0,1120) is complete after slab 13 but issued from the ACT queue after slab
# 16's broadcast so its data does not displace load traffic; store2
# [1120,1280) needs the last slab and goes on the by-then-idle sync ring.
STORE1_END = 1120

XBUFS, SBUFS = 6, 3

_FP32 = mybir.dt.float32

_cache = {}


def _build_bass(reps: int = 1):
    """Build the single-core Bass program (same NEFF runs SPMD on all cores).

    reps > 1 wraps the pipeline in a tc.For_i loop repeating it on the same
    data — used only for slope-based HW timing in test.py.
    """
    nc = bacc.Bacc("TRN2", target_bir_lowering=False, debug=False,
                   num_devices=N_CORES)

    x_d = nc.dram_tensor("x", (ELEMS_PER_CORE,), _FP32, kind="ExternalInput").ap()
    w_d = nc.dram_tensor("w", (1, KW * C), _FP32, kind="ExternalInput").ap()
    out_d = nc.dram_tensor("out", (OUT_ELEMS_PER_CORE,), _FP32,
                           kind="ExternalOutput").ap()

    x128 = x_d.rearrange("(p f) -> p f", p=128)
    o128 = out_d.rearrange("(p f) -> p f", p=128)

    with tile.TileContext(nc) as tc:
        with (
            tc.tile_pool(name="const", bufs=1) as cpool,
            tc.tile_pool(name="x", bufs=XBUFS) as xpool,
            tc.tile_pool(name="sc", bufs=2) as scpool,
            tc.tile_pool(name="s", bufs=SBUFS) as spool,
            tc.tile_pool(name="ob", bufs=2) as obpool,
            tc.tile_pool(name="psum", bufs=1,
                         space=bass.MemorySpace.PSUM) as ppool,
        ):
            # Weights ship as (1, 640) = 2.5 KB and are replicated to all
            # 128 partitions via a PE outer product (ones^T @ w) so the
            # 327 KB tiled constant never touches the DMA engines (which
            # run at the HBM wall).  PE/ACT are idle at t=0; the replicate
            # completes well inside the first slab's 7 us load shadow.
            w1 = cpool.tile([1, KW * C], _FP32)
            ones = cpool.tile([1, 128], _FP32)
            wt = cpool.tile([128, KW * C], _FP32)
            nc.scalar.dma_start(w1[:], w_d)
            nc.vector.memset(ones[:], 1.0)
            pw1 = ppool.tile([128, 512], _FP32, tag="pw1")
            pw2 = ppool.tile([128, KW * C - 512], _FP32, tag="pw2")
            nc.tensor.matmul(pw1[:], ones[:], w1[:, 0:512],
                             start=True, stop=True)
            nc.tensor.matmul(pw2[:], ones[:], w1[:, 512:KW * C],
                             start=True, stop=True)
            nc.scalar.copy(wt[:, 0:512], pw1[:])
            nc.scalar.copy(wt[:, 512:KW * C], pw2[:])

            def body():
                ob = obpool.tile([128, OUT_PER_PART], _FP32, tag="ob")
                off = 0
                for idx, f in enumerate(SIZES):
                    wpp = f // (KW * C)  # windows in this slab per partition
                    oo = off // KW       # out offset within ob

                    xt = xpool.tile([128, max(SIZES)], _FP32, tag="x")
                    nc.sync.dma_start(xt[:, :f], x128[:, off:off + f])

                    # fused channel-sum + lag-weight + lag-sum, one custom
                    # DVE op per window: accum_out = sum(x_window * wfull)
                    st = spool.tile([128, max(SIZES) // (KW * C)],
                                    _FP32, tag="s")
                    sc = scpool.tile([128, KW * C], _FP32, tag="sc")
                    for t in range(wpp):
                        nc.vector.affine_mul_reduce(
                            sc[:],
                            st[:, t:t + 1],
                            xt[:, t * KW * C:(t + 1) * KW * C],
                            wt[:],
                            1.0, 0.0)

                    # broadcast to 10 channels into the persistent out block
                    nc.scalar.copy(
                        ob[:, oo:oo + wpp * C].rearrange(
                            "p (t c) -> p t c", c=C),
                        st[:, :wpp].unsqueeze(2).broadcast_to([128, wpp, C]))

                    if idx == len(SIZES) - 2:
                        nc.scalar.dma_start(
                            o128[:, 0:STORE1_END], ob[:, 0:STORE1_END])
                    elif idx == len(SIZES) - 1:
                        nc.sync.dma_start(
                            o128[:, STORE1_END:], ob[:, STORE1_END:])
                    off += f

            if reps > 1:
                with tc.For_i(0, reps, 1):
                    body()
            else:
                body()

    nc.compile()
    return nc


def _softmax_w(param5, param6):
    i = np.arange(1, KW + 1, dtype=np.float32)
    ll = np.float32(param5) * i + np.float32(param6) * i * i
    ll = ll - ll.max()
    e = np.exp(ll)
    return (e / e.sum()).astype(np.float32)


def _weights(param5, param6):
    wfull = np.repeat(_softmax_w(param5, param6), C)      # (640,) = w[j//10]
    return wfull[None, :].copy()                          # (1, 640)


def kernel(x: np.ndarray, param5: np.ndarray, param6: np.ndarray):
    x = np.ascontiguousarray(x, dtype=np.float32)
    assert x.shape == (B, T, C)

    if "nc" not in _cache:
        _cache["nc"] = _build_bass()
    nc = _cache["nc"]

    w_tiled = _weights(param5, param6)
    shards = x.reshape(N_CORES, ELEMS_PER_CORE)
    in_maps = [{"x": shards[c], "w": w_tiled} for c in range(N_CORES)]

    res = run_bass_kernel_spmd(nc, in_maps, core_ids=list(range(N_CORES)))
    _cache["last_results"] = res

    out = np.empty((B, NWIN, C), dtype=np.float32)
    for c in range(N_CORES):
        out[c * B_PER_CORE:(c + 1) * B_PER_CORE] = (
            res.results[c]["out"].reshape(B_PER_CORE, NWIN, C))
    return out
